# revision 27
# baseline (speedup 1.0000x reference)
"""Trainium2 Bass kernel: CausalCrossConditionalSelfAttention (fp16/bf16).

Reference (B=4, T=1536, C=768, H=12, D=64):
    q/k/v = x @ W{q,k,v}.T + b   -> heads [B,H,T,64]
    att   = softmax(mask(q k^T / 8)),  mask = tile(tril(512), (3,3))
    y     = att @ v;  out = y @ Wp.T + bp

Sharding (8 cores): data-parallel over B (4) x tensor-parallel over heads in
2 groups of 6.  Each core emits a partial [1536, 768] output; the host sums
the two head-group partials per batch (and adds bp there).

Numerics: operands fp16, attention scores bf16, all matmul accumulation in
fp32 PSUM; exact exp on ACT; measured rel err ~2e-3 (budget 2e-2).

Performance structure (TRN2: PE drops to 1.2 GHz after any idle gap, needs
3us continuous execution for 2.4 GHz):
  - head-PAIR phasing: projections for pair p+1 are emitted as PE filler
    inside pair p's attention steps, so PE stays continuously busy.
  - S^T per (key-block a, key-tile m) at 128-token granularity of the
    block-causal mask; PSUM slots packed [m0 | m1+m3 | m2] -> 3 ACT exp ops
    and 3 DVE bf16 mask multiplies per (qb, head) step.
  - AV with es stationary ([128 keys, 128 q] tiles) and v moving -> output
    y in natural [token, d] orientation with the softmax denominator in an
    extra ones-column of v; denominators land on token partitions, so
    normalization is a 4-element reciprocal + per-partition-scalar multiply.
  - normalized y transposed back via paired permutation matmuls (fp16 PSUM
    bitcast), evacuated at DVE 2x 16-bit rate; per-qb output projection
    starts as soon as its last head finishes; outputs DMA'd per token tile.
"""

import math
from contextlib import ExitStack

import numpy as np
import ml_dtypes

import concourse.bass as bass
import concourse.bacc as bacc
import concourse.mybir as mybir
import concourse.tile as tile
from concourse.bass_utils import run_bass_kernel_spmd

F32 = mybir.dt.float32
F16 = mybir.dt.float16
BF16 = mybir.dt.bfloat16
AF = mybir.ActivationFunctionType

B, T, C = 4, 1536, 768
H = 12
D = 64
NCORES = 8
HG = H // 2        # heads per core (6)
CL = HG * D        # local channels per core (384)
NP = 3             # head pairs per core
VW = D + 1         # v tile width incl. ones column
SCALE = 1.0 / math.sqrt(D)
# es column layout within one key-block a (1280 cols): m0@0, m1@512, m3@896,
# m2@1024; AV slice start for (m, mq) = ESBASE[m] + 128*mq
ESBASE = (0, 384, 768, 512)


def build_nc():
    nc = bacc.Bacc("TRN2", target_bir_lowering=False, debug=False,
                   enable_asserts=False)

    xt_d = nc.dram_tensor("xt", [128, 6 * T], F16, kind="ExternalInput").ap()
    wq_d = nc.dram_tensor("wq", [128, 6 * CL], F16, kind="ExternalInput").ap()
    wk_d = nc.dram_tensor("wk", [128, 6 * CL], F16, kind="ExternalInput").ap()
    wv_d = nc.dram_tensor("wv", [128, 6 * CL], F16, kind="ExternalInput").ap()
    wp_d = nc.dram_tensor("wp", [128, 3 * C], F16, kind="ExternalInput").ap()
    bq_d = nc.dram_tensor("bq", [128, 3], F32, kind="ExternalInput").ap()
    bk_d = nc.dram_tensor("bk", [128, 3], F32, kind="ExternalInput").ap()
    bvr_d = nc.dram_tensor("bvr", [1, CL], F32, kind="ExternalInput").ap()
    mask_d = nc.dram_tensor("maskt", [128, 768], BF16, kind="ExternalInput").ap()
    eye_d = nc.dram_tensor("eye", [128, 128], F16, kind="ExternalInput").ap()
    ones_d = nc.dram_tensor("ones", [1, 128], F32, kind="ExternalInput").ap()
    out_d = nc.dram_tensor("out", [T, C], F32, kind="ExternalOutput").ap()

    with tile.TileContext(nc) as tc, ExitStack() as ctx, \
            nc.allow_low_precision(reason="fp16/bf16 pipeline, 2e-2 budget"):
        constp = ctx.enter_context(tc.tile_pool(name="constp", bufs=1))
        xwp = ctx.enter_context(tc.tile_pool(name="xwp", bufs=1))
        qkp = ctx.enter_context(tc.tile_pool(name="qkp", bufs=1))
        esp = ctx.enter_context(tc.tile_pool(name="esp", bufs=1))
        ynp = ctx.enter_context(tc.tile_pool(name="ynp", bufs=1))
        iop = ctx.enter_context(tc.tile_pool(name="iop", bufs=1))
        sps = ctx.enter_context(tc.tile_pool(name="sps", bufs=2, space="PSUM"))
        psq = ctx.enter_context(tc.tile_pool(name="psq", bufs=2, space="PSUM"))

        # ---- input DMAs: small constants first (they unblock PE/DVE heads),
        # then xt column-major across SP+gpsimd queues so the first
        # projection's six K-chunks all land ~1.5us in; weights on the
        # scalar queue (ACT idle at start) ----
        ones1 = constp.tile([1, 128], F32)
        nc.sync.dma_start(ones1[:], ones_d[:])
        bvr = constp.tile([1, CL], F32)
        nc.gpsimd.dma_start(bvr[:], bvr_d[:])
        bq32 = constp.tile([128, 3], F32)
        nc.gpsimd.dma_start(bq32[:], bq_d[:])
        bk32 = constp.tile([128, 3], F32)
        nc.gpsimd.dma_start(bk32[:], bk_d[:])
        wk16 = xwp.tile([128, 6 * CL], F16)
        for p in range(3):
            nc.scalar.dma_start(wk16[:, p * 768:(p + 1) * 768],
                                wk_d[:, p * 768:(p + 1) * 768])
        wq16 = xwp.tile([128, 6 * CL], F16)
        nc.scalar.dma_start(wq16[:], wq_d[:])
        xt16 = xwp.tile([128, 6 * T], F16)
        for nt in range(3):
            for kc in range(6):
                eng = nc.sync if kc % 2 == 0 else nc.gpsimd
                eng.dma_start(
                    xt16[:, kc * T + nt * 512:kc * T + nt * 512 + 512],
                    xt_d[:, kc * T + nt * 512:kc * T + nt * 512 + 512])
        wv16 = xwp.tile([128, 6 * CL], F16)
        nc.scalar.dma_start(wv16[:], wv_d[:])
        wp16 = xwp.tile([128, 3 * C], F16)
        nc.scalar.dma_start(wp16[:], wp_d[:])
        maskt = constp.tile([128, 768], BF16)
        nc.sync.dma_start(maskt[:], mask_d[:])
        eye16 = constp.tile([128, 128], F16)
        nc.sync.dma_start(eye16[:], eye_d[:])

        bv_bc = constp.tile([128, CL], F32)

        # ---- persistent activations ----
        qT = qkp.tile([128, NP * T], F16)   # [64*j+d, p*1536 + t]
        kT = qkp.tile([128, NP * T], F16)
        vaug = qkp.tile([128, 12 * HG * VW], BF16)  # col = tt*390 + hh*65 + d
        yT = qkp.tile([128, NP * T], F16)
        # ones column of vaug (written once; v evacs only touch cols 0:64)
        nc.vector.memset(
            vaug[:].rearrange("p (t w) -> p t w", w=VW)[:, :, D:VW], 1.0)

        es_bufs = [esp.tile([128, 3 * 1280], BF16, name=f"es{i}")
                   for i in range(3)]

        # ---- emission helpers ----
        def proj_qk_group(w16, dstT, bias32, p, nt):
            ps = psq.tile([128, 512], F32, tag="pq", name=f"qk{p}{nt}")
            for kc in range(6):
                nc.tensor.matmul(ps[:],
                                 w16[:, p * 768 + kc * 128:p * 768 + kc * 128 + 128],
                                 xt16[:, kc * T + nt * 512:kc * T + nt * 512 + 512],
                                 start=(kc == 0), stop=(kc == 5))
            nc.vector.tensor_scalar_add(
                dstT[:, p * T + nt * 512:p * T + nt * 512 + 512], ps[:],
                bias32[:, p:p + 1])

        def proj_v_group(p, tt):
            ps = psq.tile([128, 512], F32, tag="pq", name=f"v{p}{tt}")
            for kc in range(6):
                nc.tensor.matmul(ps[:, 0:128],
                                 xt16[:, kc * T + tt * 128:kc * T + tt * 128 + 128],
                                 wv16[:, p * 768 + kc * 128:p * 768 + kc * 128 + 128],
                                 start=(kc == 0), stop=(kc == 5))
            base = tt * HG * VW + 2 * p * VW
            dst = vaug[:, base:base + 2 * VW].rearrange(
                "p (h w) -> p h w", w=VW)[:, :, 0:D]
            src = ps[:, 0:128].rearrange("p (h w) -> p h w", w=D)
            bvv = bv_bc[:, 2 * p * D:2 * p * D + 128].rearrange(
                "p (h w) -> p h w", w=D)
            nc.vector.tensor_add(dst, src, bvv)

        def emit_S(p, qb, j):
            rows = slice(64 * j, 64 * j + 64)
            kc0 = p * T
            qc0 = p * T + qb * 512
            sA = sps.tile([128, 1536], F32, tag="sp", name="sA")
            for a in range(3):
                nc.tensor.matmul(
                    sA[:, a * 512:a * 512 + 512],
                    kT[rows, kc0 + a * 512:kc0 + a * 512 + 128],
                    qT[rows, qc0:qc0 + 512], start=True, stop=True)
            sB = sps.tile([128, 1536], F32, tag="sp", name="sB")
            for a in range(3):
                nc.tensor.matmul(
                    sB[:, a * 512:a * 512 + 384],
                    kT[rows, kc0 + a * 512 + 128:kc0 + a * 512 + 256],
                    qT[rows, qc0 + 128:qc0 + 512], start=True, stop=True)
            for a in range(3):
                nc.tensor.matmul(
                    sB[:, a * 512 + 384:a * 512 + 512],
                    kT[rows, kc0 + a * 512 + 384:kc0 + a * 512 + 512],
                    qT[rows, qc0 + 384:qc0 + 512], start=False, stop=True,
                    skip_group_check=True)
            return sA, sB

        def emit_S2(p, qb, j):
            rows = slice(64 * j, 64 * j + 64)
            kc0 = p * T
            qc0 = p * T + qb * 512
            sC = sps.tile([128, 1536], F32, tag="sp", name="sC")
            for a in range(3):
                nc.tensor.matmul(
                    sC[:, a * 512:a * 512 + 256],
                    kT[rows, kc0 + a * 512 + 256:kc0 + a * 512 + 384],
                    qT[rows, qc0 + 256:qc0 + 512], start=True, stop=True)
            return sC

        def exp_op(es, src, s_w, e_off, e_w):
            nc.scalar.activation(
                es[:].rearrange("p (a c) -> p a c", c=1280)[:, :, e_off:e_off + e_w],
                src[:].rearrange("p (a c) -> p a c", c=512)[:, :, 0:s_w],
                AF.Exp, scale=SCALE)

        def mask_op(es, off, w, eng=None):
            v = es[:].rearrange("p (a c) -> p a c", c=1280)[:, :, off:off + w]
            m = maskt[:].rearrange("p (a c) -> p a c", c=w)[:, 0:3, :]
            (eng or nc.vector).tensor_mul(v, v, m)

        def emit_AV(s):
            p, qb, j = steps[s]
            hh = 2 * p + j
            es = es_bufs[s % 3]
            av = psq.tile([128, 512], F32, tag="pq", name="av")
            for mq in range(4):
                for a in range(3):
                    for m in range(mq + 1):
                        col = a * 1280 + ESBASE[m] + 128 * mq
                        nc.tensor.matmul(
                            av[:, 65 * mq:65 * mq + 65],
                            es[:, col:col + 128],
                            vaug[:, (4 * a + m) * HG * VW + hh * VW:
                                 (4 * a + m) * HG * VW + hh * VW + VW],
                            start=(mq == 0 and a == 0 and m == 0),
                            stop=(a == 2 and m == mq), skip_group_check=True)
            recl = ynp.tile([128, 4], F32, tag="recl", bufs=3, name="recl")
            nc.vector.reciprocal(
                recl[:].rearrange("p (t o) -> p t o", o=1),
                av[:, 0:260].rearrange("p (t w) -> p t w", w=VW)[:, :, D:VW])
            yn = ynp.tile([128, 256], F16, tag="yn", bufs=3, name="yn")
            nc.vector.tensor_mul(
                yn[:].rearrange("p (t w) -> p t w", w=D),
                av[:, 0:260].rearrange("p (t w) -> p t w", w=VW)[:, :, 0:D],
                recl[:].rearrange("p (t o) -> p t o", o=1)
                .broadcast_to((128, 4, D)))
            return yn

        def emit_T(s, yn):
            p, qb, j = steps[s]
            tp = sps.tile([128, 1536], F32, tag="sp", name="tp")
            tpv = tp[:].bitcast(F16)
            for t in range(2):
                off = t * 1024  # separate PSUM banks
                nc.tensor.matmul(tpv[0:128, off:off + 128],
                                 yn[:, 128 * t:128 * t + 128], eye16[:],
                                 is_transpose=True, start=True, stop=True)
                dst0 = p * T + qb * 512 + 256 * t
                nc.vector.tensor_copy(
                    yT[64 * j:64 * j + 64, dst0:dst0 + 128],
                    tpv[0:64, off:off + 128])
                nc.vector.tensor_copy(
                    yT[64 * j:64 * j + 64, dst0 + 128:dst0 + 256],
                    tpv[64:128, off:off + 128])

        def outproj_group(qb, mtl):
            mt = qb * 4 + mtl
            osb = iop.tile([128, C], F32, tag="osb", bufs=3, name="osb")
            for no, w in ((0, 512), (512, 256)):
                ps = psq.tile([128, 512], F32, tag="pq", name=f"o{mt}{no}")
                for kc in range(3):
                    nc.tensor.matmul(
                        ps[:, 0:w],
                        yT[:, kc * T + mt * 128:kc * T + mt * 128 + 128],
                        wp16[:, kc * C + no:kc * C + no + w],
                        start=(kc == 0), stop=(kc == 2))
                if qb < 2 or mt % 2 == 0:
                    nc.vector.tensor_copy(osb[:, no:no + w], ps[:, 0:w])
                else:
                    nc.scalar.copy(osb[:, no:no + w], ps[:, 0:w])
            if qb < 2:
                eng = (nc.sync, nc.gpsimd)[mt % 2]
            else:
                eng = (nc.sync, nc.scalar)[mt % 2]
            eng.dma_start(out_d[mt * 128:mt * 128 + 128, :], osb[:])

        # ---- schedule ----
        steps = [(p, qb, j) for p in range(NP) for qb in range(3)
                 for j in range(2)]

        # filler work per step index
        filler = {s: [] for s in range(18)}
        for p in (1, 2):
            base = (p - 1) * 6
            for i, (w16, dstT, b32) in enumerate(
                    ((wq16, qT, bq32), (wk16, kT, bk32))):
                for nt in range(3):
                    filler[base + i * 3 + nt].append(
                        lambda w=w16, d=dstT, b=b32, pp=p, n=nt:
                        proj_qk_group(w, d, b, pp, n))
            for tt in range(12):
                filler[base + tt // 2].append(
                    lambda pp=p, t=tt: proj_v_group(pp, t))
        filler[15] += [lambda: outproj_group(0, 0), lambda: outproj_group(0, 1)]
        filler[16] += [lambda: outproj_group(0, 2), lambda: outproj_group(0, 3)]
        filler[17] += [lambda: outproj_group(1, 0), lambda: outproj_group(1, 1)]
        tail_out = [(1, 2), (1, 3), (2, 0), (2, 1), (2, 2), (2, 3)]

        # pair 0 projections (dense PE warmup); bv broadcast (K=1 ones
        # matmul) emitted after the first q-proj groups so it doesn't
        # head-of-line-block the PE queue on its input DMAs
        filler[0] = ([lambda nt=nt: proj_qk_group(wq16, qT, bq32, 0, nt)
                      for nt in (1, 2)]
                     + [lambda t=tt: proj_v_group(0, t) for tt in range(12)]
                     + filler[0])
        bv_ps = psq.tile([128, 512], F32, tag="pq", name="bv_ps")
        nc.tensor.matmul(bv_ps[:, 0:CL], ones1[:, 0:128], bvr[:],
                         start=True, stop=True)
        nc.vector.tensor_copy(bv_bc[:], bv_ps[:, 0:CL])
        for nt in range(3):
            proj_qk_group(wk16, kT, bk32, 0, nt)
        proj_qk_group(wq16, qT, bq32, 0, 0)

        for s in range(18):
            p, qb, j = steps[s]
            es = es_bufs[s % 3]
            sA, sB = emit_S(p, qb, j)
            if s >= 1:
                yn = emit_AV(s - 1)
                emit_T(s - 1, yn)
            exp_op(es, sA, 512, 0, 512)
            exp_op(es, sB, 512, 512, 512)
            mask_op(es, 0, 128, nc.gpsimd)
            mask_op(es, 512, 128, nc.gpsimd)
            for fn in filler[s]:
                fn()
            sC = emit_S2(p, qb, j)
            exp_op(es, sC, 256, 1024, 256)
            mask_op(es, 896, 256, nc.gpsimd)
        yn = emit_AV(17)
        emit_T(17, yn)
        for qb, mtl in tail_out:
            outproj_group(qb, mtl)

    nc.compile()
    return nc


_NC_CACHE = None


def _get_nc():
    global _NC_CACHE
    if _NC_CACHE is None:
        _NC_CACHE = build_nc()
    return _NC_CACHE


def make_in_maps(inputs):
    x = np.asarray(inputs["x"], dtype=np.float32)
    wq = np.asarray(inputs["Wq"], np.float32)
    wk = np.asarray(inputs["Wk"], np.float32)
    wv = np.asarray(inputs["Wv"], np.float32)
    wp = np.asarray(inputs["Wp"], np.float32)
    bq = np.asarray(inputs["bq"], np.float32)
    bk = np.asarray(inputs["bk"], np.float32)
    bv = np.asarray(inputs["bv"], np.float32)
    consts = {
        "maskt": np.tile(np.triu(np.ones((128, 128), np.float32)),
                         (1, 6)).astype(ml_dtypes.bfloat16),
        "eye": np.eye(128, dtype=np.float16),
        "ones": np.ones((1, 128), dtype=np.float32),
    }

    def wlayout(wT, kchunks, mwidth):
        # [K, M] -> [128, kchunks * M] with col = kc*M + m
        return np.ascontiguousarray(
            wT.reshape(kchunks, 128, mwidth).transpose(1, 0, 2)
            .reshape(128, kchunks * mwidth).astype(np.float16))

    def wlayout_pmajor(wT):
        # [768, 384] -> [128, 2304] with col = p*768 + kc*128 + ch
        return np.ascontiguousarray(
            wT.reshape(6, 128, 3, 128).transpose(1, 2, 0, 3)
            .reshape(128, 2304).astype(np.float16))

    in_maps = []
    for c in range(NCORES):
        b, g = c // 2, c % 2
        sl = slice(g * CL, (g + 1) * CL)
        m = {
            "xt": wlayout(x[b].T, 6, T),
            "wq": wlayout_pmajor(wq[sl].T),
            "wk": wlayout_pmajor(wk[sl].T),
            "wv": wlayout_pmajor(wv[sl].T),
            "wp": wlayout(wp[:, sl].T, 3, C),
            "bq": np.ascontiguousarray(bq[sl].reshape(3, 128).T),
            "bk": np.ascontiguousarray(bk[sl].reshape(3, 128).T),
            "bvr": np.ascontiguousarray(bv[sl].reshape(1, CL)),
            **consts,
        }
        in_maps.append(m)
    return in_maps


def combine_outputs(results, bp):
    out = np.empty((B, T, C), dtype=np.float32)
    for b in range(B):
        out[b] = results[2 * b]["out"] + results[2 * b + 1]["out"] + bp
    return out


def kernel(**inputs):
    nc = _get_nc()
    res = run_bass_kernel_spmd(nc, make_in_maps(inputs),
                               core_ids=list(range(NCORES)))
    return combine_outputs(res.results, np.asarray(inputs["bp"], np.float32))
